# revision 76
# baseline (speedup 1.0000x reference)
"""Masked dot-product attention on 8 TRN2 NeuronCores.

Math (per batch b):
    S = Q @ K^T / sqrt(64)                    [SQ, SK]
    S[:, k >= vl_b] = -1e6; A = softmax(S)    (masked cols -> weight 0)
    O = A @ V                                 [SQ, 64]

Device strategy (per core, SPMD — identical instruction stream):
  * scores are computed transposed: S_T[k, q] = sum_d K[k,d] Q[q,d]
    via matmul(lhsT=K^T tile [64,128], rhs=Q^T chunk [64,512]).
  * no max-subtraction: |S/8| <= ~9 so exp never overflows; the
    reference's masked lanes underflow to exactly 0 in fp32, we instead
    zero V rows (host-side) so masked keys contribute 0 to both
    numerator and denominator — identical result, zero device masking
    cost.
  * exp runs on BOTH the scalar (ACT) engine and the vector (DVE)
    engine.  ACT does exact exp at (N+352)/1.2 ns; DVE approximates via
    a one-op Schraudolph: i16 = rni(S*A + B) whose bit pattern IS
    f16(exp(S/8)) up to the mantissa-linearization curve (~2% RMS).
    B includes the classic bias correction (C~38) so approximated
    k-tiles are not systematically over-weighted vs exact tiles in the
    softmax ratio.  ~38% of units go to DVE: measured end-to-end
    rel-fro error ~0.8e-2 (gate 2e-2).  Splitting exp across two
    engines removes the single-ACT roofline that gated the baseline.
  * engine assignment is boundary-aware: the first pair of every slot
    is ACT-only, so the slot-end PSUM->SBUF output copies (which sit in
    the DVE queue and wait on the slot's last mm2) never head-of-line
    block the next slot's first DVE exp.
  * denominator via ones-column appended to V (host-side):
    O_aug^T[65, q] = sum_k V_aug[k, :]^T * P[k, q] accumulated in PSUM
    over k-tiles; row 64 is the softmax denominator.
  * host does final divide + transpose (tiny); outputs ship as bf16
    (halves output DMA; +1e-4 rel error, inside tolerance).
  * matmul operands are fp16 (PE streams 2-byte dtypes at full rate).
    PSUM accumulation stays fp32.
  * ~9 dummy matmuls run during the initial DMA fill so the PE activity
    monitor (HAM) sees sustained activity and releases the 1.2 GHz
    cold-clock throttle before real work arrives; the first input DMAs
    are prioritized (later prefetches gated on the first matmul) so
    compute starts ~4us earlier than with free-running round-robin DMA.

Work scheduling: the host knows valid_lens at compile time, so each core
receives a host-packed list of (q-chunk "slot", k-tile "unit") work items
covering only k < vl. All cores run the same program shape (same slot/unit
counts, compile-time constants); per-core differences live entirely in the
packed input data. Cores with fewer real k-tiles get padding units whose
V_aug is all-zero (contributes nothing).
"""

import functools
import math

import numpy as np

B, SQ, SK, D = 16, 2048, 2048, 64
NCORES = 8
KT = 128          # k rows per unit (one matmul stationary tile)
QCH = 1024        # q columns per slot
NSLOTS_TOTAL = B * (SQ // QCH)   # 32 slot-items across all cores
SLOTS_PER_CORE = NSLOTS_TOTAL // NCORES  # 4
VA_W = D + 1      # V columns + ones column
# uin dram row: K^T pair cols (128) + va_A (65) + va_B (65), tightly packed
UW_DR = KT + 2 * VA_W  # 258
# uin SBUF row: adds 63 once-zeroed pad cols so both mm2 weight loads can be
# full 128-wide (full-width keeps the PE HAM activity high; a half-idle
# array throttles the clock to 1.2 GHz).  lhsT_A = cols 128:256 (va_A +
# va_B head as ignored out-partitions 65:128), lhsT_B = cols 193:321
# (va_B + zero pad).
UW_SB = KT + 2 * VA_W + (KT - VA_W)  # 321

# Schraudolph exp on DVE: bits = rni(s*EXP_A + EXP_B) viewed as f16 is
# exp(s/8) up to mantissa linearization.  s is the RAW score (scale 1/8
# folded into EXP_A).  |s| < 90 keeps bits in (0, 32767): no NaN/saturate.
# The -38 bias-centers the (1+f)/2^f curve so approx tiles aren't
# systematically over-weighted relative to exact (ACT) tiles.
EXP_A = 1024.0 * math.log2(math.e) / 8.0
EXP_B = 15.0 * 1024.0 - 38.0

N_WARM = 10       # HAM pre-warm dummy matmuls: back-to-back 512-col
                  # matmuls are exactly the dense activity the HAM's
                  # ~3.4us busy-window needs, so the clock is already at
                  # 2.4 GHz when real work starts (~10.8us); real work at
                  # warm pace also needs its uin stream ~2us later,
                  # absorbing occasional per-core fabric hiccups

_last_results = None  # stashed BassKernelResults for test.py introspection


def _nkt(vl: int) -> int:
    return max(1, min(SK // KT, math.ceil(vl / KT)))


def _make_schedule(vl: np.ndarray, full: bool = False):
    """Assign the 32 (batch, q-half) slot-items to 8 cores, balanced by
    k-tile count. An item may be SPLIT across slots/cores (partial-k
    attention sums are additive; the host sums partial outputs before
    dividing), which lets slot sizes drop below their group max with the
    overflow going to shared spill slots.

    Returns (slot_sizes, assign): slot_sizes[s] is the compile-time unit
    count of slot s (identical on every core); assign[core][s] is
    (batch, half, k_tile_start, n_real_ktiles) or None (pure padding)."""
    w = [SK // KT if full else _nkt(int(vl[b])) for b in range(B)]
    items = sorted(((b, h) for b in range(B) for h in range(SQ // QCH)),
                   key=lambda t: -w[t[0]])
    ngroups = len(items) // NCORES  # 4
    groups = [items[NCORES * s : NCORES * s + NCORES] for s in range(ngroups)]
    gmax = [max(w[b] for b, _ in g) for g in groups]
    gmin = [min(w[b] for b, _ in g) for g in groups]

    def evaluate(p):
        leftovers = []  # (len, batch, half, k_start)
        for s, g in enumerate(groups):
            for b, h in g:
                if w[b] > p[s]:
                    leftovers.append((w[b] - p[s], b, h, p[s]))
        leftovers.sort(key=lambda t: -t[0])
        spares = []
        for i in range(0, len(leftovers), NCORES):
            spares.append(leftovers[i : i + NCORES])
        spare_sizes = [chunk[0][0] for chunk in spares]
        return sum(p) + sum(spare_sizes), spares, spare_sizes

    import itertools
    best = None
    ranges = [range(gmin[s], gmax[s] + 1) for s in range(ngroups)]
    # keep the search tractable: only consider the top few reductions
    ranges = [r if len(r) <= 8 else range(gmax[s] - 7, gmax[s] + 1)
              for s, r in zip(range(ngroups), ranges)]
    for p in itertools.product(*ranges):
        total, spares, spare_sizes = evaluate(list(p))
        # each slot adds a pipeline-boundary stall worth ~0.7 units
        cost = total + 0.7 * (len(p) + len(spares))
        if best is None or cost < best[0]:
            best = (cost, list(p), spares, spare_sizes)
    _, p, spares, spare_sizes = best

    slot_sizes = list(p) + spare_sizes
    assign = [[None] * len(slot_sizes) for _ in range(NCORES)]
    for s, g in enumerate(groups):
        for c, (b, h) in enumerate(g):
            assign[c][s] = (b, h, 0, min(w[b], p[s]))
    for k, chunk in enumerate(spares):
        for c, (ln, b, h, k_start) in enumerate(chunk):
            assign[c][ngroups + k] = (b, h, k_start, ln)
    # slot order: a medium slot opens (one Q chunk + a few uin tiles carry
    # the whole HAM clock ramp), the small slots run mid-kernel where
    # prefetch is several pairs ahead and their boundary stalls are
    # cheapest, and the largest slot closes as one long saturated stretch
    # (no boundary stalls near the tail)
    by_size = sorted(range(len(slot_sizes)), key=lambda s: -slot_sizes[s])
    order = by_size[1:] + [by_size[0]]
    slot_sizes = [slot_sizes[s] for s in order]
    assign = [[a[s] for s in order] for a in assign]
    return tuple(slot_sizes), assign


@functools.lru_cache(maxsize=4)
def _build_program(slot_sizes: tuple):
    """Build + schedule the SPMD Bass program for the given slot shape."""
    import concourse.bacc as bacc
    import concourse.mybir as mybir
    import concourse.tile as tile

    n_units = sum(slot_sizes)
    f32 = mybir.dt.float32
    f16 = mybir.dt.float16
    bf16 = mybir.dt.bfloat16
    i16 = mybir.dt.int16

    nc = bacc.Bacc(
        "TRN2",
        target_bir_lowering=False,
        debug=False,
        enable_asserts=False,
        num_devices=NCORES,
    )
    n_pairs = sum((u + 1) // 2 for u in slot_sizes)  # slot-local pairing
    n_slots = len(slot_sizes)
    qtd = nc.dram_tensor("qtd", [n_slots, KT, QCH], f16, kind="ExternalInput")
    uin = nc.dram_tensor("uin", [n_pairs, KT, UW_DR], f16, kind="ExternalInput")
    o = nc.dram_tensor("o", [n_slots, VA_W, QCH], bf16, kind="ExternalOutput")

    with tile.TileContext(nc) as tc:
        with (
            tc.tile_pool(name="qpool", bufs=4) as qpool,
            tc.tile_pool(name="upool", bufs=5) as upool,
            tc.tile_pool(name="ptpool", bufs=4) as ptpool,
            tc.tile_pool(name="opool", bufs=2) as opool,
            tc.tile_pool(name="wpool", bufs=1) as wpool,
            tc.tile_pool(name="scpool", bufs=1, space="PSUM") as scpool,
            tc.tile_pool(name="accpool", bufs=1, space="PSUM") as accpool,
        ):
            # Per pair of k-tile units (A, B): the 4 mm1 matmuls are emitted
            # interleaved (A-c0, B-c0, A-c1, B-c1) on PE row groups h0/h64 so
            # the two 64-deep contractions execute CONCURRENTLY in the array.
            # This both halves mm1 time and keeps array activity high enough
            # for the HAM clock gate to run the PE at full clock (a K=64
            # half-array stream alone stays throttled at 1.2 GHz).
            #
            # PE queue order is pinned to
            #   ... mm1-pair(p) -> mm2-pair(p-1) -> mm1-pair(p+1) ...
            # so the previous pair's mm2 fills the exp latency. Score tiles
            # rotate through 3 single-buffered PSUM tags (6 banks, +2 for the
            # accumulator = all 8), giving mm1 three units of WAR slack
            # against exp.
            scale = 1.0 / math.sqrt(D)
            exp_f = mybir.ActivationFunctionType.Exp
            # Dummy exp with no dependencies: pulls the ~2.7us ACT table
            # load into the DMA-priming phase instead of the first real exp.
            warm = qpool.tile([1, 8], f32, name="warm", tag="warm")
            nc.vector.memset(warm, 0.0)
            nc.scalar.activation(warm, warm, exp_f, scale=1.0)
            # HAM pre-warm: dummy matmuls over a zeroed tile keep the PE
            # array active while the first input DMAs land, so the clock
            # monitor ramps to full speed before real work starts.
            wsrc = wpool.tile([KT, 5 * KT], f16, name="wsrc", tag="wsrc")
            nc.vector.memset(wsrc, 0.0)
            # NOTE: the uin pad columns (UW_DR:UW_SB) are never written —
            # both mm2 weight loads map them to out-partitions 65:128 whose
            # accumulator rows are never copied out, so stale SBUF garbage
            # (even NaN) there is harmless and a zeroing pass would only
            # delay the first uin DMA behind its WAW dependency.
            wacc = accpool.tile([KT, QCH], f32, name="acc")
            warm_mms = []
            for i in range(N_WARM):
                warm_mms.append(nc.tensor.matmul(
                    wacc[:, 0:512],
                    lhsT=wsrc[:, 0:KT],
                    rhs=wsrc[:, KT : KT + 512],
                    start=True,
                    stop=True,
                ))
            for a, b in zip(warm_mms, warm_mms[1:]):
                tile.add_dep_helper(b.ins, a.ins, False, "pe order")
            pending = []      # mm2 calls of the previous pair (emitted,
                              # ordering deferred until next pair's mm1s)
            prev_mm2_last = warm_mms[-1]  # last PE instr of the prior group
            gu = 0   # unit counter (sc-tag rotation)
            p_idx = 0  # global pair counter (uin index)
            n_elig = 0  # jp>=1 pairs seen (A-side DVE rebalance counter)
            udmas = []  # uin DMA handles (head fabric gating)
            for s, nu in enumerate(slot_sizes):
                # DMA queue routing: per-queue issue cost (~0.7us) and FIFO
                # order make queue choice matter.  uin rides the Sync hwdge
                # queue (first-needed, strictly ordered); qtd rides gpsimd's
                # software DGE (prefetched a slot ahead, latency-tolerant);
                # slot 0's qtd is split across the scalar hwdge + gpsimd
                # queues so the three first transfers use separate queues
                # and compute starts ~3us earlier.
                qt = qpool.tile([KT, QCH], f16)
                if s == 0:
                    nc.scalar.dma_start(out=qt[0:D, :], in_=qtd[0, 0:D])
                    nc.gpsimd.dma_start(out=qt[D:KT, :], in_=qtd[0, D:KT])
                elif s == n_slots - 1:
                    # the closing (largest) slot's Q is the most
                    # latency-exposed prefetch: two parallel pieces halve
                    # its transfer time
                    nc.gpsimd.dma_start(out=qt[0:D, :], in_=qtd[s, 0:D])
                    nc.gpsimd.dma_start(out=qt[D:KT, :], in_=qtd[s, D:KT])
                else:
                    nc.gpsimd.dma_start(out=qt, in_=qtd[s])
                acc = accpool.tile([KT, QCH], f32, name="acc")
                for jp in range((nu + 1) // 2):
                    ump = upool.tile([KT, UW_SB], f16, name="ump")
                    if p_idx <= 2:
                        # two concurrent pieces halve the early transfers'
                        # latency (one DMA's packets stream ~40 GB/s); these
                        # gate the pipeline start and the HAM clock ramp.
                        # Second halves ride the scalar/gpsimd queues so all
                        # pieces are issued by ~7.7us — margin against the
                        # occasional per-core fabric hiccup.
                        eng2 = (nc.sync, nc.scalar, nc.gpsimd)[p_idx]
                        nc.sync.dma_start(out=ump[0:D, 0:UW_DR],
                                          in_=uin[p_idx, 0:D])
                        udma = eng2.dma_start(out=ump[D:KT, 0:UW_DR],
                                              in_=uin[p_idx, D:KT])
                    else:
                        udma = nc.sync.dma_start(out=ump[:, 0:UW_DR],
                                                 in_=uin[p_idx])
                    udmas.append(udma)
                    p_idx += 1
                    # Exp engine per unit: first pair of a slot is DVE-only
                    # (the slot-end copies now sit in the ACT queue waiting
                    # on the slot's last mm2 and would otherwise head-of-line
                    # block the next slot's first ACT exp); later pairs send
                    # A to ACT and most B's to DVE (~57% DVE share overall,
                    # which balances ACT = exps + copies vs DVE = exps).
                    if jp == 0:
                        # interior slots: both exps on DVE so the ACT-queue
                        # copy can't block them.  Slot 0 has no preceding
                        # copy, so split A/B across engines — the serial
                        # 2x1224ns DVE burst otherwise stalls every core's
                        # pipeline start by ~1.2us.
                        dve_half = {0: s != 0, 1: True}
                    else:
                        dve_half = {0: False, 1: n_elig % 4 != 3}
                        n_elig += 1
                    # A lone unit still gets a dummy row-group-B partner for
                    # mm1 (zero V_aug, no exp/mm2): a half-array matmul
                    # stream drops the HAM activity metric and re-throttles
                    # the PE clock to 1.2 GHz.
                    lone = 2 * jp + 1 >= nu
                    units = []
                    for half in (0, 1):
                        j = 2 * jp + half
                        real = not (lone and half == 1)
                        rows = slice(0, D) if half == 0 else slice(D, KT)
                        units.append((
                            j,
                            real,
                            dve_half[half],
                            ump[rows, 0:KT],                     # K^T tile
                            qt[rows, :],                          # Q^T stream
                            ump[:, KT + half * VA_W : KT + half * VA_W + KT],
                            scpool.tile([KT, QCH], f32, name=f"sc_{gu}_{half}",
                                        tag=f"sc{(gu + half) % 3}"),
                            ptpool.tile([KT, QCH], f16, name=f"pt_{gu}_{half}",
                                        tag=f"pt{half}") if real else None,
                        ))
                    mm1 = []
                    nchunk = QCH // 512
                    for c in range(nchunk):
                        for j, real, dve, kt_t, qt_h, va_t, sc, pt in units:
                            mm1.append(nc.tensor.matmul(
                                sc[:, c * 512 : (c + 1) * 512],
                                lhsT=kt_t,
                                rhs=qt_h[:, c * 512 : (c + 1) * 512],
                                start=True,
                                stop=True,
                            ))
                            # emit each unit's exp right after its last mm1
                            # chunk so its engine-queue wait lands per-exp.
                            # The program's very first unit runs its exp
                            # per-512-chunk on BOTH engines concurrently:
                            # its score tile's WAR gates pair 1's mm1 during
                            # pipeline fill, and chunking frees it ~0.5us
                            # sooner (one bubble per kernel, every core).
                            if s == 0 and jp == 0 and j == 0:
                                cs = slice(c * 512, (c + 1) * 512)
                                if c == 0:
                                    nc.scalar.activation(pt[:, cs], sc[:, cs],
                                                         exp_f, scale=scale)
                                else:
                                    nc.vector.tensor_scalar(
                                        pt.bitcast(i16)[:, cs], sc[:, cs],
                                        EXP_A, EXP_B,
                                        mybir.AluOpType.mult,
                                        mybir.AluOpType.add)
                            elif c == nchunk - 1 and real:
                                if dve:
                                    nc.vector.tensor_scalar(
                                        pt.bitcast(i16), sc, EXP_A, EXP_B,
                                        mybir.AluOpType.mult,
                                        mybir.AluOpType.add)
                                else:
                                    nc.scalar.activation(pt, sc, exp_f,
                                                         scale=scale)
                    if prev_mm2_last is not None:
                        tile.add_dep_helper(mm1[0].ins, prev_mm2_last.ins,
                                            False, "pe order")
                    for a, b in zip(mm1, mm1[1:]):
                        tile.add_dep_helper(b.ins, a.ins, False, "pe order")
                    for mm2 in pending:
                        tile.add_dep_helper(mm2.ins, mm1[-1].ins, False,
                                            "mm2 after next pair's mm1")
                    prev_mm2_last = pending[-1] if pending else prev_mm2_last
                    pending = []
                    # the very last pair's mm2s go chunk-major so the c0
                    # accumulator region completes two matmuls earlier and
                    # the tail's first output copy can start sooner
                    final_pair = (s == len(slot_sizes) - 1
                                  and jp == (nu + 1) // 2 - 1)
                    mm2_order = (
                        [(c, u) for c in range(QCH // 512) for u in units]
                        if final_pair else
                        [(c, u) for u in units for c in range(QCH // 512)])
                    for c, (j, real, dve, kt_t, qt_h, va_t, sc, pt) in mm2_order:
                        if not real:
                            continue
                        pending.append(nc.tensor.matmul(
                            acc[:, c * 512 : (c + 1) * 512],
                            lhsT=va_t,
                            rhs=pt[:, c * 512 : (c + 1) * 512],
                            start=(j == 0),
                            stop=(j == nu - 1),
                        ))
                    for a, b in zip(pending, pending[1:]):
                        tile.add_dep_helper(b.ins, a.ins, False, "pe order")
                    gu += 2
                # copy + store per 512-col half so the first half streams out
                # while the slot's last mm2 still writes the second half.
                # Copies live on DVE (whose next-slot exp work starts only at
                # pair 1, so the acc-wait can't block it); the final slot's
                # second half goes to the now-idle ACT engine to cut the tail.
                o_sb = opool.tile([VA_W, QCH], bf16)
                last = s == len(slot_sizes) - 1
                if not last:
                    # one wide copy on ACT (the less-loaded exp engine, and
                    # off DVE's queue so it can't delay exp_B); the store is
                    # latency-tolerant, so it rides gpsimd's queue to keep
                    # Sync clear for uin
                    nc.scalar.activation(
                        o_sb, acc[0:VA_W, :],
                        mybir.ActivationFunctionType.Copy)
                    nc.gpsimd.dma_start(out=o[s], in_=o_sb)
                else:
                    # tail: four 256-col pieces alternate DVE/ACT and the
                    # stores alternate Sync/Scalar queues, pipelining
                    # copy -> issue -> transfer so the kernel end isn't one
                    # serial chain
                    for c in range(4):
                        src = acc[0:VA_W, c * 256 : (c + 1) * 256]
                        dst = o_sb[:, c * 256 : (c + 1) * 256]
                        if c % 2 == 0:
                            nc.vector.tensor_copy(dst, src)
                        else:
                            nc.scalar.activation(
                                dst, src, mybir.ActivationFunctionType.Copy)
                        oeng = nc.sync if c % 2 == 0 else nc.gpsimd
                        oeng.dma_start(out=o[s, :, c * 256 : (c + 1) * 256],
                                       in_=o_sb[:, c * 256 : (c + 1) * 256])
    nc.compile()
    return nc


def _pack_inputs(queries, keys, values, vl, slot_sizes, assign):
    """Build each core's packed device inputs per its schedule (mirrors the
    device program's slot-local pairing exactly)."""
    n_pairs = sum((u + 1) // 2 for u in slot_sizes)
    n_slots = len(slot_sizes)
    qT = np.ascontiguousarray(queries.transpose(0, 2, 1).astype(np.float16))
    kT = keys.astype(np.float16)  # [B, SK, D] row-major, sliced per k-tile
    in_maps = []
    for c in range(NCORES):
        qtd = np.zeros((n_slots, KT, QCH), np.float16)
        uin = np.zeros((n_pairs, KT, UW_DR), np.float16)
        p_idx = 0
        for s, nu in enumerate(slot_sizes):
            if assign[c][s] is None:
                p_idx += (nu + 1) // 2
                continue  # pure-padding slot: all-zero inputs contribute 0
            b, h, ks, w = assign[c][s]
            qtd[s, :D] = qT[b, :, h * QCH : (h + 1) * QCH]
            qtd[s, D:KT] = qtd[s, :D]  # duplicate for the h64 row half
            nvalid = int(vl[b])
            for jp in range((nu + 1) // 2):
                for half in (0, 1):
                    # a lone unit's B half is a dummy mm1 partner (device
                    # skips its exp/mm2): real K data keeps array activity up
                    j = min(2 * jp + half, nu - 1)
                    t = ks + min(j, w - 1)  # padding units replay a k-tile
                    rows = slice(0, D) if half == 0 else slice(D, KT)
                    uin[p_idx, rows, :KT] = kT[b, t * KT : (t + 1) * KT, :].T
                    if j < w and not (half == 1 and 2 * jp + 1 >= nu):
                        k0 = t * KT
                        nv = min(max(nvalid - k0, 0), KT)
                        col0 = KT + half * VA_W
                        uin[p_idx, :nv, col0 : col0 + D] = values[b, k0 : k0 + nv, :]
                        uin[p_idx, :nv, col0 + D] = 1.0
                    # padding units leave V_aug zero -> contribute nothing
                p_idx += 1
        in_maps.append({"qtd": qtd, "uin": uin})
    return in_maps


def kernel(queries, keys, values, valid_lens, _full=False, _trace=False):
    global _last_results
    from concourse.bass_utils import run_bass_kernel_spmd

    queries = np.ascontiguousarray(np.asarray(queries, dtype=np.float32))
    keys = np.ascontiguousarray(np.asarray(keys, dtype=np.float32))
    values = np.ascontiguousarray(np.asarray(values, dtype=np.float32))
    vl = np.asarray(valid_lens).astype(np.int64).reshape(B)

    slot_sizes, assign = _make_schedule(vl, full=_full)
    nc = _build_program(slot_sizes)
    in_maps = _pack_inputs(queries, keys, values, vl, slot_sizes, assign)

    kwargs = {"trace": True} if _trace else {}
    res = run_bass_kernel_spmd(nc, in_maps, core_ids=list(range(NCORES)), **kwargs)
    _last_results = res

    # Sum partial (numerator, denominator) contributions per (batch, q-half),
    # then divide once — exact for split items.
    agg = np.zeros((B, SQ // QCH, VA_W, QCH), np.float64)
    for c in range(NCORES):
        o = np.asarray(res.results[c]["o"]).astype(np.float64)
        for s in range(len(slot_sizes)):
            if assign[c][s] is None:
                continue
            b, h, _, _ = assign[c][s]
            agg[b, h] += o[s]
    out = np.empty((B, SQ, D), np.float32)
    for b in range(B):
        for h in range(SQ // QCH):
            num = agg[b, h, :D, :]
            den = agg[b, h, D, :]
            out[b, h * QCH : (h + 1) * QCH, :] = (num / den).T.astype(np.float32)
    return out


# revision 77
# speedup vs baseline: 1.0572x; 1.0572x over previous
"""Masked dot-product attention on 8 TRN2 NeuronCores.

Math (per batch b):
    S = Q @ K^T / sqrt(64)                    [SQ, SK]
    S[:, k >= vl_b] = -1e6; A = softmax(S)    (masked cols -> weight 0)
    O = A @ V                                 [SQ, 64]

Device strategy (per core, SPMD — identical instruction stream):
  * scores are computed transposed: S_T[k, q] = sum_d K[k,d] Q[q,d]
    via matmul(lhsT=K^T tile [64,128], rhs=Q^T chunk [64,512]).
  * no max-subtraction: |S/8| <= ~9 so exp never overflows; the
    reference's masked lanes underflow to exactly 0 in fp32, we instead
    zero V rows (host-side) so masked keys contribute 0 to both
    numerator and denominator — identical result, zero device masking
    cost.
  * exp runs on BOTH the scalar (ACT) engine and the vector (DVE)
    engine.  ACT does exact exp at (N+352)/1.2 ns; DVE approximates via
    a one-op Schraudolph: i16 = rni(S*A + B) whose bit pattern IS
    f16(exp(S/8)) up to the mantissa-linearization curve (~2% RMS).
    B includes the classic bias correction (C~38) so approximated
    k-tiles are not systematically over-weighted vs exact tiles in the
    softmax ratio.  ~38% of units go to DVE: measured end-to-end
    rel-fro error ~0.8e-2 (gate 2e-2).  Splitting exp across two
    engines removes the single-ACT roofline that gated the baseline.
  * engine assignment is boundary-aware: the first pair of every slot
    is ACT-only, so the slot-end PSUM->SBUF output copies (which sit in
    the DVE queue and wait on the slot's last mm2) never head-of-line
    block the next slot's first DVE exp.
  * denominator via ones-column appended to V (host-side):
    O_aug^T[65, q] = sum_k V_aug[k, :]^T * P[k, q] accumulated in PSUM
    over k-tiles; row 64 is the softmax denominator.
  * host does final divide + transpose (tiny); outputs ship as bf16
    (halves output DMA; +1e-4 rel error, inside tolerance).
  * matmul operands are fp16 (PE streams 2-byte dtypes at full rate).
    PSUM accumulation stays fp32.
  * ~9 dummy matmuls run during the initial DMA fill so the PE activity
    monitor (HAM) sees sustained activity and releases the 1.2 GHz
    cold-clock throttle before real work arrives; the first input DMAs
    are prioritized (later prefetches gated on the first matmul) so
    compute starts ~4us earlier than with free-running round-robin DMA.

Work scheduling: the host knows valid_lens at compile time, so each core
receives a host-packed list of (q-chunk "slot", k-tile "unit") work items
covering only k < vl. All cores run the same program shape (same slot/unit
counts, compile-time constants); per-core differences live entirely in the
packed input data. Cores with fewer real k-tiles get padding units whose
V_aug is all-zero (contributes nothing).
"""

import functools
import math

import numpy as np

B, SQ, SK, D = 16, 2048, 2048, 64
NCORES = 8
KT = 128          # k rows per unit (one matmul stationary tile)
QCH = 1024        # q columns per slot
NSLOTS_TOTAL = B * (SQ // QCH)   # 32 slot-items across all cores
SLOTS_PER_CORE = NSLOTS_TOTAL // NCORES  # 4
VA_W = D + 1      # V columns + ones column
# uin dram row: K^T pair cols (128) + va_A (65) + va_B (65), tightly packed
UW_DR = KT + 2 * VA_W  # 258
# uin SBUF row: adds 63 once-zeroed pad cols so both mm2 weight loads can be
# full 128-wide (full-width keeps the PE HAM activity high; a half-idle
# array throttles the clock to 1.2 GHz).  lhsT_A = cols 128:256 (va_A +
# va_B head as ignored out-partitions 65:128), lhsT_B = cols 193:321
# (va_B + zero pad).
UW_SB = KT + 2 * VA_W + (KT - VA_W)  # 321

# Schraudolph exp on DVE: bits = rni(s*EXP_A + EXP_B) viewed as f16 is
# exp(s/8) up to mantissa linearization.  s is the RAW score (scale 1/8
# folded into EXP_A).  |s| < 90 keeps bits in (0, 32767): no NaN/saturate.
# The -38 bias-centers the (1+f)/2^f curve so approx tiles aren't
# systematically over-weighted relative to exact (ACT) tiles.
EXP_A = 1024.0 * math.log2(math.e) / 8.0
EXP_B = 15.0 * 1024.0 - 38.0

N_WARM = 10       # HAM pre-warm dummy matmuls: back-to-back 512-col
                  # matmuls are exactly the dense activity the HAM's
                  # ~3.4us busy-window needs, so the clock is already at
                  # 2.4 GHz when real work starts (~10.8us); real work at
                  # warm pace also needs its uin stream ~2us later,
                  # absorbing occasional per-core fabric hiccups

_last_results = None  # stashed BassKernelResults for test.py introspection


def _nkt(vl: int) -> int:
    return max(1, min(SK // KT, math.ceil(vl / KT)))


def _make_schedule(vl: np.ndarray, full: bool = False):
    """Assign the 32 (batch, q-half) slot-items to 8 cores, balanced by
    k-tile count. An item may be SPLIT across slots/cores (partial-k
    attention sums are additive; the host sums partial outputs before
    dividing), which lets slot sizes drop below their group max with the
    overflow going to shared spill slots.

    Returns (slot_sizes, assign): slot_sizes[s] is the compile-time unit
    count of slot s (identical on every core); assign[core][s] is
    (batch, half, k_tile_start, n_real_ktiles) or None (pure padding)."""
    w = [SK // KT if full else _nkt(int(vl[b])) for b in range(B)]
    items = sorted(((b, h) for b in range(B) for h in range(SQ // QCH)),
                   key=lambda t: -w[t[0]])
    ngroups = len(items) // NCORES  # 4
    groups = [items[NCORES * s : NCORES * s + NCORES] for s in range(ngroups)]
    gmax = [max(w[b] for b, _ in g) for g in groups]
    gmin = [min(w[b] for b, _ in g) for g in groups]

    def evaluate(p):
        leftovers = []  # (len, batch, half, k_start)
        for s, g in enumerate(groups):
            for b, h in g:
                if w[b] > p[s]:
                    leftovers.append((w[b] - p[s], b, h, p[s]))
        leftovers.sort(key=lambda t: -t[0])
        spares = []
        for i in range(0, len(leftovers), NCORES):
            spares.append(leftovers[i : i + NCORES])
        spare_sizes = [chunk[0][0] for chunk in spares]
        return sum(p) + sum(spare_sizes), spares, spare_sizes

    import itertools
    best = None
    ranges = [range(gmin[s], gmax[s] + 1) for s in range(ngroups)]
    # keep the search tractable: only consider the top few reductions
    ranges = [r if len(r) <= 8 else range(gmax[s] - 7, gmax[s] + 1)
              for s, r in zip(range(ngroups), ranges)]
    for p in itertools.product(*ranges):
        total, spares, spare_sizes = evaluate(list(p))
        # each slot adds a pipeline-boundary stall worth ~0.7 units
        cost = total + 0.7 * (len(p) + len(spares))
        if best is None or cost < best[0]:
            best = (cost, list(p), spares, spare_sizes)
    _, p, spares, spare_sizes = best

    slot_sizes = list(p) + spare_sizes
    assign = [[None] * len(slot_sizes) for _ in range(NCORES)]
    for s, g in enumerate(groups):
        for c, (b, h) in enumerate(g):
            assign[c][s] = (b, h, 0, min(w[b], p[s]))
    for k, chunk in enumerate(spares):
        for c, (ln, b, h, k_start) in enumerate(chunk):
            assign[c][ngroups + k] = (b, h, k_start, ln)
    # slot order: a medium slot opens (one Q chunk + a few uin tiles carry
    # the whole HAM clock ramp), the small slots run mid-kernel where
    # prefetch is several pairs ahead and their boundary stalls are
    # cheapest, and the largest slot closes as one long saturated stretch
    # (no boundary stalls near the tail)
    by_size = sorted(range(len(slot_sizes)), key=lambda s: -slot_sizes[s])
    order = by_size[1:] + [by_size[0]]
    slot_sizes = [slot_sizes[s] for s in order]
    assign = [[a[s] for s in order] for a in assign]
    return tuple(slot_sizes), assign


@functools.lru_cache(maxsize=4)
def _build_program(slot_sizes: tuple):
    """Build + schedule the SPMD Bass program for the given slot shape."""
    import concourse.bacc as bacc
    import concourse.mybir as mybir
    import concourse.tile as tile

    n_units = sum(slot_sizes)
    f32 = mybir.dt.float32
    f16 = mybir.dt.float16
    bf16 = mybir.dt.bfloat16
    i16 = mybir.dt.int16

    nc = bacc.Bacc(
        "TRN2",
        target_bir_lowering=False,
        debug=False,
        enable_asserts=False,
        num_devices=NCORES,
    )
    n_pairs = sum((u + 1) // 2 for u in slot_sizes)  # slot-local pairing
    n_slots = len(slot_sizes)
    qtd = nc.dram_tensor("qtd", [n_slots, KT, QCH], f16, kind="ExternalInput")
    uin = nc.dram_tensor("uin", [n_pairs, KT, UW_DR], f16, kind="ExternalInput")
    o = nc.dram_tensor("o", [n_slots, VA_W, QCH], bf16, kind="ExternalOutput")

    with tile.TileContext(nc) as tc:
        with (
            tc.tile_pool(name="qpool", bufs=4) as qpool,
            tc.tile_pool(name="upool", bufs=5) as upool,
            tc.tile_pool(name="ptpool", bufs=4) as ptpool,
            tc.tile_pool(name="opool", bufs=2) as opool,
            tc.tile_pool(name="wpool", bufs=1) as wpool,
            tc.tile_pool(name="scpool", bufs=1, space="PSUM") as scpool,
            tc.tile_pool(name="accpool", bufs=1, space="PSUM") as accpool,
        ):
            # Per pair of k-tile units (A, B): the 4 mm1 matmuls are emitted
            # interleaved (A-c0, B-c0, A-c1, B-c1) on PE row groups h0/h64 so
            # the two 64-deep contractions execute CONCURRENTLY in the array.
            # This both halves mm1 time and keeps array activity high enough
            # for the HAM clock gate to run the PE at full clock (a K=64
            # half-array stream alone stays throttled at 1.2 GHz).
            #
            # PE queue order is pinned to
            #   ... mm1-pair(p) -> mm2-pair(p-1) -> mm1-pair(p+1) ...
            # so the previous pair's mm2 fills the exp latency. Score tiles
            # rotate through 3 single-buffered PSUM tags (6 banks, +2 for the
            # accumulator = all 8), giving mm1 three units of WAR slack
            # against exp.
            scale = 1.0 / math.sqrt(D)
            exp_f = mybir.ActivationFunctionType.Exp
            # Dummy exp with no dependencies: pulls the ~2.7us ACT table
            # load into the DMA-priming phase instead of the first real exp.
            warm = qpool.tile([1, 8], f32, name="warm", tag="warm")
            nc.vector.memset(warm, 0.0)
            nc.scalar.activation(warm, warm, exp_f, scale=1.0)
            # HAM pre-warm: dummy matmuls over a zeroed tile keep the PE
            # array active while the first input DMAs land, so the clock
            # monitor ramps to full speed before real work starts.
            wsrc = wpool.tile([KT, 5 * KT], f16, name="wsrc", tag="wsrc")
            nc.vector.memset(wsrc, 0.0)
            # NOTE: the uin pad columns (UW_DR:UW_SB) are never written —
            # both mm2 weight loads map them to out-partitions 65:128 whose
            # accumulator rows are never copied out, so stale SBUF garbage
            # (even NaN) there is harmless and a zeroing pass would only
            # delay the first uin DMA behind its WAW dependency.
            wacc = accpool.tile([KT, QCH], f32, name="acc")
            warm_mms = []
            for i in range(N_WARM):
                warm_mms.append(nc.tensor.matmul(
                    wacc[:, 0:512],
                    lhsT=wsrc[:, 0:KT],
                    rhs=wsrc[:, KT : KT + 512],
                    start=True,
                    stop=True,
                ))
            for a, b in zip(warm_mms, warm_mms[1:]):
                tile.add_dep_helper(b.ins, a.ins, False, "pe order")
            pending = []      # mm2 calls of the previous pair (emitted,
                              # ordering deferred until next pair's mm1s)
            prev_mm2_last = warm_mms[-1]  # last PE instr of the prior group
            gu = 0   # unit counter (sc-tag rotation)
            p_idx = 0  # global pair counter (uin index)
            n_elig = 0  # jp>=1 pairs seen (A-side DVE rebalance counter)
            udmas = []  # uin DMA handles (head fabric gating)
            for s, nu in enumerate(slot_sizes):
                # DMA queue routing: per-queue issue cost (~0.7us) and FIFO
                # order make queue choice matter.  uin rides the Sync hwdge
                # queue (first-needed, strictly ordered); qtd rides gpsimd's
                # software DGE (prefetched a slot ahead, latency-tolerant);
                # slot 0's qtd is split across the scalar hwdge + gpsimd
                # queues so the three first transfers use separate queues
                # and compute starts ~3us earlier.
                qt = qpool.tile([KT, QCH], f16)
                if s == 0:
                    nc.scalar.dma_start(out=qt[0:D, :], in_=qtd[0, 0:D])
                    nc.gpsimd.dma_start(out=qt[D:KT, :], in_=qtd[0, D:KT])
                elif s == n_slots - 1:
                    # the closing (largest) slot's Q is the most
                    # latency-exposed prefetch: two parallel pieces halve
                    # its transfer time
                    nc.gpsimd.dma_start(out=qt[0:D, :], in_=qtd[s, 0:D])
                    nc.gpsimd.dma_start(out=qt[D:KT, :], in_=qtd[s, D:KT])
                else:
                    nc.gpsimd.dma_start(out=qt, in_=qtd[s])
                acc = accpool.tile([KT, QCH], f32, name="acc")
                for jp in range((nu + 1) // 2):
                    ump = upool.tile([KT, UW_SB], f16, name="ump")
                    if p_idx <= 2:
                        # two concurrent pieces halve the early transfers'
                        # latency (one DMA's packets stream ~40 GB/s); these
                        # gate the pipeline start and the HAM clock ramp.
                        # Second halves ride the scalar/gpsimd queues so all
                        # pieces are issued by ~7.7us — margin against the
                        # occasional per-core fabric hiccup.
                        eng2 = (nc.sync, nc.scalar, nc.gpsimd)[p_idx]
                        nc.sync.dma_start(out=ump[0:D, 0:UW_DR],
                                          in_=uin[p_idx, 0:D])
                        udma = eng2.dma_start(out=ump[D:KT, 0:UW_DR],
                                              in_=uin[p_idx, D:KT])
                    else:
                        udma = nc.sync.dma_start(out=ump[:, 0:UW_DR],
                                                 in_=uin[p_idx])
                    udmas.append(udma)
                    p_idx += 1
                    # Exp engine per unit: first pair of a slot is DVE-only
                    # (the slot-end copies now sit in the ACT queue waiting
                    # on the slot's last mm2 and would otherwise head-of-line
                    # block the next slot's first ACT exp); later pairs send
                    # A to ACT and most B's to DVE (~57% DVE share overall,
                    # which balances ACT = exps + copies vs DVE = exps).
                    if jp == 0:
                        # interior slots: both exps on DVE so the ACT-queue
                        # copy can't block them.  Slot 0 has no preceding
                        # copy, so split A/B across engines — the serial
                        # 2x1224ns DVE burst otherwise stalls every core's
                        # pipeline start by ~1.2us.
                        dve_half = {0: s != 0, 1: True}
                    else:
                        dve_half = {0: False, 1: n_elig % 4 != 3}
                        n_elig += 1
                    # A lone unit still gets a dummy row-group-B partner for
                    # mm1 (zero V_aug, no exp/mm2): a half-array matmul
                    # stream drops the HAM activity metric and re-throttles
                    # the PE clock to 1.2 GHz.
                    lone = 2 * jp + 1 >= nu
                    units = []
                    for half in (0, 1):
                        j = 2 * jp + half
                        real = not (lone and half == 1)
                        rows = slice(0, D) if half == 0 else slice(D, KT)
                        units.append((
                            j,
                            real,
                            dve_half[half],
                            ump[rows, 0:KT],                     # K^T tile
                            qt[rows, :],                          # Q^T stream
                            ump[:, KT + half * VA_W : KT + half * VA_W + KT],
                            scpool.tile([KT, QCH], f32, name=f"sc_{gu}_{half}",
                                        tag=f"sc{(gu + half) % 3}"),
                            ptpool.tile([KT, QCH], f16, name=f"pt_{gu}_{half}",
                                        tag=f"pt{half}") if real else None,
                        ))
                    mm1 = []
                    nchunk = QCH // 512
                    for c in range(nchunk):
                        for j, real, dve, kt_t, qt_h, va_t, sc, pt in units:
                            mm1.append(nc.tensor.matmul(
                                sc[:, c * 512 : (c + 1) * 512],
                                lhsT=kt_t,
                                rhs=qt_h[:, c * 512 : (c + 1) * 512],
                                start=True,
                                stop=True,
                            ))
                            # emit each unit's exp right after its last mm1
                            # chunk so its engine-queue wait lands per-exp
                            if c == nchunk - 1 and real:
                                if dve:
                                    nc.vector.tensor_scalar(
                                        pt.bitcast(i16), sc, EXP_A, EXP_B,
                                        mybir.AluOpType.mult,
                                        mybir.AluOpType.add)
                                else:
                                    nc.scalar.activation(pt, sc, exp_f,
                                                         scale=scale)
                    if prev_mm2_last is not None:
                        tile.add_dep_helper(mm1[0].ins, prev_mm2_last.ins,
                                            False, "pe order")
                    for a, b in zip(mm1, mm1[1:]):
                        tile.add_dep_helper(b.ins, a.ins, False, "pe order")
                    for mm2 in pending:
                        tile.add_dep_helper(mm2.ins, mm1[-1].ins, False,
                                            "mm2 after next pair's mm1")
                    prev_mm2_last = pending[-1] if pending else prev_mm2_last
                    pending = []
                    # the very last pair's mm2s go chunk-major so the c0
                    # accumulator region completes two matmuls earlier and
                    # the tail's first output copy can start sooner
                    final_pair = (s == len(slot_sizes) - 1
                                  and jp == (nu + 1) // 2 - 1)
                    mm2_order = (
                        [(c, u) for c in range(QCH // 512) for u in units]
                        if final_pair else
                        [(c, u) for u in units for c in range(QCH // 512)])
                    for c, (j, real, dve, kt_t, qt_h, va_t, sc, pt) in mm2_order:
                        if not real:
                            continue
                        pending.append(nc.tensor.matmul(
                            acc[:, c * 512 : (c + 1) * 512],
                            lhsT=va_t,
                            rhs=pt[:, c * 512 : (c + 1) * 512],
                            start=(j == 0),
                            stop=(j == nu - 1),
                        ))
                    for a, b in zip(pending, pending[1:]):
                        tile.add_dep_helper(b.ins, a.ins, False, "pe order")
                    gu += 2
                # copy + store per 512-col half so the first half streams out
                # while the slot's last mm2 still writes the second half.
                # Copies live on DVE (whose next-slot exp work starts only at
                # pair 1, so the acc-wait can't block it); the final slot's
                # second half goes to the now-idle ACT engine to cut the tail.
                o_sb = opool.tile([VA_W, QCH], bf16)
                last = s == len(slot_sizes) - 1
                if not last:
                    # one wide copy on ACT (the less-loaded exp engine, and
                    # off DVE's queue so it can't delay exp_B); the store is
                    # latency-tolerant, so it rides gpsimd's queue to keep
                    # Sync clear for uin
                    nc.scalar.activation(
                        o_sb, acc[0:VA_W, :],
                        mybir.ActivationFunctionType.Copy)
                    nc.gpsimd.dma_start(out=o[s], in_=o_sb)
                else:
                    # tail: four 256-col pieces alternate DVE/ACT and the
                    # stores alternate Sync/Scalar queues, pipelining
                    # copy -> issue -> transfer so the kernel end isn't one
                    # serial chain
                    for c in range(4):
                        src = acc[0:VA_W, c * 256 : (c + 1) * 256]
                        dst = o_sb[:, c * 256 : (c + 1) * 256]
                        if c % 2 == 0:
                            nc.vector.tensor_copy(dst, src)
                        else:
                            nc.scalar.activation(
                                dst, src, mybir.ActivationFunctionType.Copy)
                        oeng = nc.sync if c % 2 == 0 else nc.gpsimd
                        oeng.dma_start(out=o[s, :, c * 256 : (c + 1) * 256],
                                       in_=o_sb[:, c * 256 : (c + 1) * 256])
    nc.compile()
    return nc


def _pack_inputs(queries, keys, values, vl, slot_sizes, assign):
    """Build each core's packed device inputs per its schedule (mirrors the
    device program's slot-local pairing exactly)."""
    n_pairs = sum((u + 1) // 2 for u in slot_sizes)
    n_slots = len(slot_sizes)
    qT = np.ascontiguousarray(queries.transpose(0, 2, 1).astype(np.float16))
    kT = keys.astype(np.float16)  # [B, SK, D] row-major, sliced per k-tile
    in_maps = []
    for c in range(NCORES):
        qtd = np.zeros((n_slots, KT, QCH), np.float16)
        uin = np.zeros((n_pairs, KT, UW_DR), np.float16)
        p_idx = 0
        for s, nu in enumerate(slot_sizes):
            if assign[c][s] is None:
                p_idx += (nu + 1) // 2
                continue  # pure-padding slot: all-zero inputs contribute 0
            b, h, ks, w = assign[c][s]
            qtd[s, :D] = qT[b, :, h * QCH : (h + 1) * QCH]
            qtd[s, D:KT] = qtd[s, :D]  # duplicate for the h64 row half
            nvalid = int(vl[b])
            for jp in range((nu + 1) // 2):
                for half in (0, 1):
                    # a lone unit's B half is a dummy mm1 partner (device
                    # skips its exp/mm2): real K data keeps array activity up
                    j = min(2 * jp + half, nu - 1)
                    t = ks + min(j, w - 1)  # padding units replay a k-tile
                    rows = slice(0, D) if half == 0 else slice(D, KT)
                    uin[p_idx, rows, :KT] = kT[b, t * KT : (t + 1) * KT, :].T
                    if j < w and not (half == 1 and 2 * jp + 1 >= nu):
                        k0 = t * KT
                        nv = min(max(nvalid - k0, 0), KT)
                        col0 = KT + half * VA_W
                        uin[p_idx, :nv, col0 : col0 + D] = values[b, k0 : k0 + nv, :]
                        uin[p_idx, :nv, col0 + D] = 1.0
                    # padding units leave V_aug zero -> contribute nothing
                p_idx += 1
        in_maps.append({"qtd": qtd, "uin": uin})
    return in_maps


def kernel(queries, keys, values, valid_lens, _full=False, _trace=False):
    global _last_results
    from concourse.bass_utils import run_bass_kernel_spmd

    queries = np.ascontiguousarray(np.asarray(queries, dtype=np.float32))
    keys = np.ascontiguousarray(np.asarray(keys, dtype=np.float32))
    values = np.ascontiguousarray(np.asarray(values, dtype=np.float32))
    vl = np.asarray(valid_lens).astype(np.int64).reshape(B)

    slot_sizes, assign = _make_schedule(vl, full=_full)
    nc = _build_program(slot_sizes)
    in_maps = _pack_inputs(queries, keys, values, vl, slot_sizes, assign)

    kwargs = {"trace": True} if _trace else {}
    res = run_bass_kernel_spmd(nc, in_maps, core_ids=list(range(NCORES)), **kwargs)
    _last_results = res

    # Sum partial (numerator, denominator) contributions per (batch, q-half),
    # then divide once — exact for split items.
    agg = np.zeros((B, SQ // QCH, VA_W, QCH), np.float64)
    for c in range(NCORES):
        o = np.asarray(res.results[c]["o"]).astype(np.float64)
        for s in range(len(slot_sizes)):
            if assign[c][s] is None:
                continue
            b, h, _, _ = assign[c][s]
            agg[b, h] += o[s]
    out = np.empty((B, SQ, D), np.float32)
    for b in range(B):
        for h in range(SQ // QCH):
            num = agg[b, h, :D, :]
            den = agg[b, h, D, :]
            out[b, h * QCH : (h + 1) * QCH, :] = (num / den).T.astype(np.float32)
    return out
